# revision 41
# baseline (speedup 1.0000x reference)
"""BigBird regressor forward pass on 8 Trainium2 NeuronCores (Bass/Tile).

Sharding: 8 cores = batch(2) x sequence-chunks(4). Each core owns 1024 tokens
of one batch in transposed layout x^T [768, 1024]. Per layer:
  - K/V projections local, written to DRAM block-major in bf16; all Q
    projections computed up front (hidden behind the AllGather)
  - AllGather K,V (+x edge blocks) within each 4-core group
  - full attention for global blocks 0/63 split by heads across the group,
    exchanged with a small second AllGather
  - middle-block sparse attention fully local and index-driven: the host
    precomputes row-gather indices (window/global/random block slots) so every
    matmul access pattern is static; scores are computed transposed (keys on
    PSUM partitions) so no transposes are needed; V tiles carry an appended
    ones column so softmax denominators fall out of the same matmuls
  - Wo, FFN, LayerNorms are sequence-local (no more collectives).
Output: per-core scalar partial of sum_tokens(x_final . fc_w); host reduces.

Precision/perf choices (tolerance is rel 2e-2; measured ~2e-4):
  - dense projections run as float32r (1 PE cycle/col vs 4 for fp32); every
    producer tile feeding an f32r matmul is declared float32r per the BIR
    verifier's rounding rule
  - attention K/V/Q and gathered tiles are bf16 (halves AllGather + gather
    bytes); PSUM accumulation stays fp32
  - phase weights (K/V/Q/Wo) are SBUF-resident per layer, loaded with a few
    wide DMAs; FFN weights stream one [128,768] tile per fc chunk
  - GELU is a single fused Gelu_apprx_tanh activation with per-partition bias
  - K bias is dropped (constant shift per query under softmax); V bias is
    deferred to one per-partition add on the attention output
  - softmax-denominator broadcasts use K=1 matmuls against a host-loaded ones
    tile (f32r memset is invalid ISA)
Note: indirect row gathers must stay one-index-per-partition ([128,1] offset
APs); batched [128,k] offsets execute in CoreSim but corrupt data on HW.
"""
import contextlib

import numpy as np

import concourse.bass as bass
import concourse.bacc as bacc
import concourse.mybir as mybir
import concourse.tile as tile
from concourse.bass import ds

F32 = mybir.dt.float32
F32R = mybir.dt.float32r
BF16 = mybir.dt.bfloat16
I32 = mybir.dt.int32
U32 = mybir.dt.uint32
AF = mybir.ActivationFunctionType
ALU = mybir.AluOpType




B, N, D, H, L = 2, 4096, 768, 12, 2
DH, BS, NB, R, FF = 64, 64, 64, 3, 3072
DC = D // 128
FFC = FF // 128
NLOC = N // 4
TT = 512
NTT = NLOC // TT
EPS = 1e-12
SCALE = 0.125
GELU_C = float(np.sqrt(2.0 / np.pi))
GELU_A = 0.044715

SLOTS_RANK = 408
KV_ELEMS = SLOTS_RANK * 4096
EDGE_OFF = 384 * 4096
GCTX_ELEMS = 3 * 64 * 128

NSLOT = 68
WIN_W = [64, 128] + [192] * 14 + [128, 64]
WIN_QLO = [max(s - 2, 0) for s in range(18)]
WIN_QHI = [min(s, 15) for s in range(18)]
WIN_GROUPS = [[0, 1, 2], [3, 4], [5, 6], [7, 8], [9, 10], [11, 12], [13, 14],
              [15, 16, 17]]
GROUPS = [[0, 1, 2, 3], [4, 5, 6, 7]]


def build(debug=False):
    nc = bacc.Bacc("TRN2", target_bir_lowering=False, debug=False,
                   num_devices=8)

    xT0 = nc.dram_tensor("xT0", [128, DC, NLOC], F32, kind="ExternalInput")
    posT = nc.dram_tensor("posT", [128, DC, NLOC], F32, kind="ExternalInput")
    w_qkv = nc.dram_tensor("w_qkv", [L, DC, 128, 2304], F32R, kind="ExternalInput")
    w_o = nc.dram_tensor("w_o", [L, DC, 128, D], F32R, kind="ExternalInput")
    w_i = nc.dram_tensor("w_i", [L, DC, 128, FF], F32R, kind="ExternalInput")
    w_d = nc.dram_tensor("w_d", [L, FFC, 128, D], F32R, kind="ExternalInput")
    b_qk = nc.dram_tensor("b_qk", [L, 128, 12], F32, kind="ExternalInput")
    b_v = nc.dram_tensor("b_v", [L, 128, DC], F32, kind="ExternalInput")
    b_o = nc.dram_tensor("b_o", [L, 128, DC], F32, kind="ExternalInput")
    b_i = nc.dram_tensor("b_i", [L, 128, FFC], F32, kind="ExternalInput")
    b_d = nc.dram_tensor("b_d", [L, 128, DC], F32, kind="ExternalInput")
    emb_g = nc.dram_tensor("emb_g", [128, DC], F32, kind="ExternalInput")
    emb_b = nc.dram_tensor("emb_b", [128, DC], F32, kind="ExternalInput")
    ln1_g = nc.dram_tensor("ln1_g", [L, 128, DC], F32, kind="ExternalInput")
    ln1_b = nc.dram_tensor("ln1_b", [L, 128, DC], F32, kind="ExternalInput")
    ln2_g = nc.dram_tensor("ln2_g", [L, 128, DC], F32, kind="ExternalInput")
    ln2_b = nc.dram_tensor("ln2_b", [L, 128, DC], F32, kind="ExternalInput")
    fc_w = nc.dram_tensor("fc_w", [128, DC], F32, kind="ExternalInput")
    idx_k = nc.dram_tensor("idx_k", [L, 6, 128, NSLOT], I32, kind="ExternalInput")
    idx_v = nc.dram_tensor("idx_v", [L, 6, 128, NSLOT], I32, kind="ExternalInput")
    gmask = nc.dram_tensor("gmask", [128, 4], F32, kind="ExternalInput")
    idx_gk = nc.dram_tensor("idx_gk", [L, 3, 64, 64], I32, kind="ExternalInput")
    idx_gv = nc.dram_tensor("idx_gv", [L, 3, 64, 64], I32, kind="ExternalInput")
    hbase = nc.dram_tensor("hbase", [1, 1], U32, kind="ExternalInput")
    ones_in = nc.dram_tensor("ones_in", [128, 128], F32R, kind="ExternalInput")
    out_fc = nc.dram_tensor("out_fc", [1, 1], F32, kind="ExternalOutput")

    dbg = {}
    if debug:
        for nm in ("xln", "x1", "ctx0", "a0"):
            dbg[nm] = nc.dram_tensor("dbg_" + nm, [128, DC, NLOC], F32R,
                                     kind="ExternalOutput")
        dbg["q2"] = nc.dram_tensor("dbg_q2", [128, NLOC], BF16,
                                   kind="ExternalOutput")
        dbg["ksel"] = nc.dram_tensor("dbg_ksel", [128, NSLOT * 64], BF16,
                                     kind="ExternalOutput")
        dbg["vsel"] = nc.dram_tensor("dbg_vsel", [128, NSLOT * 65], BF16,
                                     kind="ExternalOutput")
        dbg["cps00"] = nc.dram_tensor("dbg_cps00", [65, 512], F32,
                                      kind="ExternalOutput")
        dbg["kva"] = nc.dram_tensor("dbg_kva", [4 * KV_ELEMS], BF16,
                                    kind="ExternalOutput")

    kv_loc = [nc.dram_tensor(f"kv_loc{l}", [KV_ELEMS], BF16) for l in range(L)]
    kv_all = [nc.dram_tensor(f"kv_all{l}", [4 * KV_ELEMS], BF16) for l in range(L)]
    gc_loc = [nc.dram_tensor(f"gc_loc{l}", [GCTX_ELEMS], F32) for l in range(L)]
    gc_all = [nc.dram_tensor(f"gc_all{l}", [4 * GCTX_ELEMS], F32) for l in range(L)]

    with tile.TileContext(nc) as tc, contextlib.ExitStack() as ex:
        pool_c = ex.enter_context(tc.tile_pool(name="consts", bufs=1))
        pool_b = ex.enter_context(tc.tile_pool(name="bigp", bufs=3))
        pool = ex.enter_context(tc.tile_pool(name="gen", bufs=1))
        pool_ps = ex.enter_context(tc.tile_pool(name="psum", bufs=4, space="PSUM"))

        def acc_ps(name):
            return pool_ps.tile([128, TT], F32, tag="acc", name=name, bufs=4)

        def ctx_ps_t(name):
            return pool_ps.tile([128, TT], F32, tag="ctx", name=name, bufs=4)

        def sb(tag, p, f, bufs, name, dt=F32):
            return pool.tile([p, f], dt, tag=tag, name=name, bufs=bufs)

        # ---------- constants ----------
        ones128 = pool_c.tile([128, 1], F32, tag="c_o128", name="ones128")
        nc.vector.memset(ones128[:], 1.0)
        ones_all = pool_c.tile([128, 128], F32R, tag="c_oall", name="ones_all")
        nc.sync.dma_start(ones_all[:], ones_in[:])
        eps_sb = pool_c.tile([128, 1], F32, tag="c_eps", name="epsc")
        nc.vector.memset(eps_sb[:], EPS)

        def ldconst(name, src, dt=F32):
            tl = pool_c.tile(list(src.shape), dt, tag=f"c_{name}", name=name)
            nc.sync.dma_start(tl[:], src[:])
            return tl

        bqk_sb = [ldconst(f"bqk{l}", b_qk[l]) for l in range(L)]
        bo_sb = [ldconst(f"bo{l}", b_o[l]) for l in range(L)]
        bi_sb = [ldconst(f"bi{l}", b_i[l]) for l in range(L)]
        bd_sb = [ldconst(f"bd{l}", b_d[l]) for l in range(L)]
        embg_sb = ldconst("embg", emb_g)
        embb_sb = ldconst("embb", emb_b)
        ln1g_sb = [ldconst(f"ln1g{l}", ln1_g[l]) for l in range(L)]
        ln1b_sb = [ldconst(f"ln1b{l}", ln1_b[l]) for l in range(L)]
        ln2g_sb = [ldconst(f"ln2g{l}", ln2_g[l]) for l in range(L)]
        ln2b_sb = [ldconst(f"ln2b{l}", ln2_b[l]) for l in range(L)]
        fcw_sb = ldconst("fcw", fc_w)
        gmask_sb = ldconst("gmask", gmask)

        bvv_sb = [ldconst(f"bvv{l}", b_v[l]) for l in range(L)]

        rv_dve = nc.vector.alloc_register("hb_dve")
        nc.vector.reg_load(rv_dve, hbase[0:1, 0:1])
        sv_dve = nc.snap(rv_dve, min_val=0, max_val=9)

        def kv_rows(l):
            return kv_all[l][:].rearrange("(r c) -> r c", c=64)

        def kv_slots(l):
            return kv_all[l][:].rearrange("(s p t) -> s p t", p=64, t=64)

        # ---------- LayerNorm (x^T layout: stats over partitions via PE) ----
        def layernorm(r_get, g_sb, b_sb, out_get):
            # stats matmuls use an all-ones [128,128] lhsT so every output
            # partition receives the column sum (pre-broadcast, no K=1 mm)
            mu_ps = pool_ps.tile([128, TT], F32, tag="ctx", name="lnmu_ps", bufs=4)
            sq_ps = pool_ps.tile([128, TT], F32, tag="ctx", name="lnsq_ps", bufs=4)
            for c in range(DC):
                nc.tensor.matmul(mu_ps[:], lhsT=(ones_all[:]), rhs=(r_get(c)),
                                 start=(c == 0), stop=(c == DC - 1))
            for c in range(DC):
                sq_sb = sb("lnt", 128, TT, 6, "lnsq", dt=F32R)
                nc.scalar.square(sq_sb[:], r_get(c))
                nc.tensor.matmul(sq_ps[:], lhsT=(ones_all[:]), rhs=(sq_sb[:]),
                                 start=(c == 0), stop=(c == DC - 1))
            mu_sb = sb("lnt", 128, TT, 6, "lnmu")
            nc.vector.tensor_scalar_mul(mu_sb[:], mu_ps[:], 1.0 / D)
            var_sb = sb("lnt", 128, TT, 6, "lnvar")
            nc.vector.tensor_scalar_mul(var_sb[:], sq_ps[:], 1.0 / D)
            mu2_sb = sb("lnt", 128, TT, 6, "lnmu2")
            nc.vector.tensor_mul(mu2_sb[:], mu_sb[:], mu_sb[:])
            nc.vector.tensor_sub(var_sb[:], var_sb[:], mu2_sb[:])
            std_sb = sb("lnt", 128, TT, 6, "lnstd")
            nc.scalar.activation(std_sb[:], var_sb[:], AF.Sqrt,
                                 bias=eps_sb[:, 0:1])
            rinv_sb = sb("lnt", 128, TT, 6, "lnrinv")
            nc.vector.reciprocal(rinv_sb[:], std_sb[:])
            for c in range(DC):
                u_sb = sb("lnt", 128, TT, 6, "lnu")
                nc.vector.tensor_sub(u_sb[:], r_get(c), mu_sb[:])
                v_sb = sb("lnt", 128, TT, 6, "lnv")
                nc.vector.scalar_tensor_tensor(v_sb[:], u_sb[:],
                                               g_sb[:, c:c + 1], rinv_sb[:],
                                               ALU.mult, ALU.mult)
                nc.vector.tensor_scalar_add(out_get(c), v_sb[:],
                                            b_sb[:, c:c + 1])

        # dense projection helper: output chunks in pairs, weights from
        # SBUF-resident phase tiles (wget(c) -> [128, ncol] AP)
        def proj_pairs(wget, rhs_get, ncol, bias_apply, nK=DC, rw=TT):
            npc = ncol // 128
            for pg in range(0, npc, 2):
                w2 = min(2, npc - pg)
                pss = [acc_ps(f"prj{pg + j}") for j in range(w2)]
                for c in range(nK):
                    wt = wget(c)
                    for j in range(w2):
                        nc.tensor.matmul(
                            pss[j][:, 0:rw],
                            lhsT=(wt[:, (pg + j) * 128:(pg + j + 1) * 128]),
                            rhs=(rhs_get(c)), start=(c == 0),
                            stop=(c == nK - 1))
                for j in range(w2):
                    bias_apply(pg + j, pss[j])

        # load one phase's worth of weights (6 chunks) into the shared
        # "wph" rotation; phases are sequential so 6 bufs suffice
        def load_wtiles(nm, src_get, ncols, n=DC):
            tiles = []
            for c in range(n):
                t = pool.tile([128, ncols], F32R, tag="wph", name=f"{nm}{c}",
                              bufs=DC)
                nc.sync.dma_start(t[:], src_get(c))
                tiles.append(t)
            return tiles

        # ---------- embeddings + LN ----------
        xT = pool_b.tile([128, DC * NLOC], F32R, tag="big", name="xT_emb")
        for tt in range(NTT):
            r_sb = sb("stA", 128, DC * TT, 1, "embr", dt=F32R)
            for c in range(DC):
                x0_sb = sb("gel", 128, TT, 4, "embx")
                nc.sync.dma_start(x0_sb[:], xT0[:, c, tt * TT:(tt + 1) * TT])
                p0_sb = sb("gel", 128, TT, 4, "embp")
                nc.sync.dma_start(p0_sb[:], posT[:, c, tt * TT:(tt + 1) * TT])
                nc.vector.tensor_add(r_sb[:, c * TT:(c + 1) * TT], x0_sb[:],
                                     p0_sb[:])
            layernorm(lambda c: r_sb[:, c * TT:(c + 1) * TT],
                      embg_sb, embb_sb,
                      lambda c: xT[:, c * NLOC + tt * TT:c * NLOC + (tt + 1) * TT])
        if debug:
            for c in range(DC):
                nc.sync.dma_start(dbg["xln"][:, c, :],
                                  xT[:, c * NLOC:(c + 1) * NLOC])

        # ---------- layers ----------
        for l in range(L):
            # ---- K projection -> kv_loc (block-major [d, t]) ----
            wk = load_wtiles("wk", lambda c: w_qkv[l, c, :, 768:1536], 768)
            for tt in range(NTT):
                def kbias(kc, ps, tt=tt):
                    st = sb("gel", 128, TT, 4, "kstage", dt=BF16)
                    nc.vector.tensor_scalar_add(st[:], ps[:],
                                                bqk_sb[l][:, 6 + kc:7 + kc])
                    for hh in range(2):
                        h = 2 * kc + hh
                        dst = bass.AP(kv_loc[l], (h * 16 + tt * 8) * 4096,
                                      [[64, 64], [4096, 8], [1, 64]])
                        nc.sync.dma_start(
                            dst, st[hh * 64:(hh + 1) * 64, :].rearrange(
                                "p (b t) -> p b t", t=64))
                proj_pairs(
                    lambda c: wk[c][:],
                    lambda c, tt=tt: xT[:, c * NLOC + tt * TT:c * NLOC + (tt + 1) * TT],
                    768, kbias)
            # ---- V projection (natural layout [tok, dd]) -> kv_loc ----
            wv = load_wtiles("wv", lambda c: w_qkv[l, c, :, 1536:2304], 768)
            for tcb in range(8):
                vst = sb("vstage", 128, 768, 2, "vstage", dt=BF16)
                for half in range(2):
                    # separate psum tiles per 256/128-col region: interleaved
                    # accumulation groups must not share a psum zero region
                    vpsa = acc_ps(f"vpsa{half}")
                    vpsb = acc_ps(f"vpsb{half}")
                    for c in range(DC):
                        nc.tensor.matmul(
                            vpsa[:, 0:256],
                            lhsT=(xT[:, c * NLOC + tcb * 128:c * NLOC + (tcb + 1) * 128]),
                            rhs=(wv[c][:, half * 384:half * 384 + 256]),
                            start=(c == 0),
                            stop=(c == DC - 1), skip_group_check=True)
                        nc.tensor.matmul(
                            vpsb[:, 0:128],
                            lhsT=(xT[:, c * NLOC + tcb * 128:c * NLOC + (tcb + 1) * 128]),
                            rhs=(wv[c][:, half * 384 + 256:(half + 1) * 384]),
                            start=(c == 0),
                            stop=(c == DC - 1), skip_group_check=True)
                    # V stored WITHOUT bias: since softmax rows sum to 1,
                    # bv is added once to ctxT after attention instead
                    nc.vector.tensor_copy(vst[:, half * 384:half * 384 + 256],
                                          vpsa[:, 0:256])
                    nc.vector.tensor_copy(vst[:, half * 384 + 256:(half + 1) * 384],
                                          vpsb[:, 0:128])
                for hh in range(2):
                    bt = 2 * tcb + hh
                    dst = bass.AP(kv_loc[l], (192 + bt) * 4096,
                                  [[64, 64], [16 * 4096, 12], [1, 64]])
                    nc.sync.dma_start(
                        dst, vst[hh * 64:(hh + 1) * 64, :].rearrange(
                            "p (h d) -> p h d", d=64))
            # ---- x edges -> kv_loc ----
            for eb in range(2):
                t0 = 0 if eb == 0 else NLOC - 64
                src = xT[:].rearrange("p (c t) -> p c t", t=NLOC)[:, :, t0:t0 + 64]
                xe_b = sb("xeb", 128, DC * 128, 2, "xeb", dt=BF16)
                nc.vector.tensor_copy(
                    xe_b[:, 0:DC * 64].rearrange("p (c t) -> p c t", t=64), src)
                dst = bass.AP(kv_loc[l], EDGE_OFF + eb * (DC * 128 * 64),
                              [[64, 128], [128 * 64, DC], [1, 64]])
                nc.sync.dma_start(
                    dst, xe_b[:, 0:DC * 64].rearrange("p (c t) -> p c t", t=64))

            # Q weights + all six head-pair Q projections run BEFORE the
            # AllGather is awaited, hiding the collective behind PE work
            wq = load_wtiles("wq", lambda c: w_qkv[l, c, :, 0:768], 768)
            q_all = sb("qall", 128, 6 * NLOC, 1, "qall", dt=BF16)
            nc.gpsimd.collective_compute(
                "AllGather", ALU.bypass, ins=[kv_loc[l][:]],
                outs=[kv_all[l][:]], replica_groups=GROUPS)
            for hp in range(6):
                for tt in range(NTT):
                    def qbias(_pg, ps, hp=hp, tt=tt):
                        nc.vector.tensor_scalar_add(
                            q_all[:, hp * NLOC + tt * TT:hp * NLOC + (tt + 1) * TT],
                            ps[:], bqk_sb[l][:, hp:hp + 1])
                    proj_pairs(
                        lambda c: wq[c][:, hp * 128:(hp + 1) * 128],
                        lambda c, tt=tt: xT[:, c * NLOC + tt * TT:c * NLOC + (tt + 1) * TT],
                        128, qbias)

            # ---- global-block (full) attention ----
            xg_b = sb("xeb", 128, DC * 128, 2, "xgb", dt=BF16)
            for eb in range(2):
                rbase = (0 if eb == 0 else 3) * KV_ELEMS
                src = bass.AP(kv_all[l], rbase + EDGE_OFF + eb * (DC * 128 * 64),
                              [[64, 128], [128 * 64, DC], [1, 64]])
                nc.sync.dma_start(
                    xg_b[:].rearrange(
                        "p (c t) -> p c t", t=128)[:, :, eb * 64:(eb + 1) * 64],
                    src)
            xg_sb = sb("stA", 128, DC * TT, 1, "xg", dt=F32R)
            nc.vector.tensor_copy(xg_sb[:, 0:DC * 128], xg_b[:])
            qg_sb = sb("qbuf", 64, 12 * 128, 1, "qg", dt=BF16)

            def qgbias(oc, ps):
                for hh in range(2):
                    h = 2 * oc + hh
                    nc.vector.tensor_scalar_add(
                        qg_sb[:, h * 128:(h + 1) * 128],
                        ps[hh * 64:(hh + 1) * 64, 0:128],
                        bqk_sb[l][hh * 64:(hh + 1) * 64, oc:oc + 1])
            proj_pairs(
                lambda c: wq[c][:],
                lambda c: xg_sb[:, c * 128:(c + 1) * 128],
                768, qgbias, rw=128)

            for i in range(3):
                kf_sb = sb("gath", 128, NSLOT * 65, 3, "kf", dt=BF16)
                va_sb = sb("gath", 128, NSLOT * 65, 3, "va", dt=BF16)
                kfull = kf_sb[0:64, 0:4096]
                vaug = va_sb[0:64, 0:64 * 65]
                igk_sb = sb("ikiv", 128, NSLOT, 4, "igk", dt=I32)
                nc.sync.dma_start(igk_sb[0:64, 0:64], idx_gk[l, i])
                igv_sb = sb("ikiv", 128, NSLOT, 4, "igv", dt=I32)
                nc.sync.dma_start(igv_sb[0:64, 0:64], idx_gv[l, i])
                for g in range(64):
                    nc.gpsimd.indirect_dma_start(
                        out=kfull[:, g * 64:(g + 1) * 64],
                        out_offset=None, in_=kv_rows(l),
                        in_offset=bass.IndirectOffsetOnAxis(
                            ap=igk_sb[0:64, g:g + 1], axis=0))
                    nc.gpsimd.indirect_dma_start(
                        out=vaug[:, g * 65:g * 65 + 64],
                        out_offset=None, in_=kv_rows(l),
                        in_offset=bass.IndirectOffsetOnAxis(
                            ap=igv_sb[0:64, g:g + 1], axis=0))
                nc.vector.memset(
                    vaug.rearrange("p (g c) -> p g c", c=65)[:, :, 64:65], 1.0)
                qgi_sb = sb("tiny", 64, 128, 3, "qgi", dt=BF16)
                nc.vector.tensor_copy(qgi_sb[:],
                                      qg_sb[:, ds((sv_dve + i) * 128, 128)])
                ctxg_ps = ctx_ps_t("ctxg")
                for gg in range(16):
                    sg_ps = acc_ps("sgF")
                    for k in range(4):
                        g = gg * 4 + k
                        nc.tensor.matmul(sg_ps[0:64, k * 128:(k + 1) * 128],
                                         lhsT=kfull[:, g * 64:(g + 1) * 64],
                                         rhs=qgi_sb[:], start=True, stop=True)
                    sg_sb = sb("sTsb", 128, TT, 2, "sgFsb", dt=BF16)
                    nc.scalar.activation(sg_sb[0:64, :], sg_ps[0:64, :],
                                         AF.Exp, scale=SCALE)
                    for k in range(4):
                        g = gg * 4 + k
                        nc.tensor.matmul(ctxg_ps[0:65, 0:128],
                                         lhsT=vaug[:, g * 65:(g + 1) * 65],
                                         rhs=sg_sb[0:64, k * 128:(k + 1) * 128],
                                         start=(g == 0), stop=(g == 63),
                                         skip_group_check=True)
                rec_sb = sb("rec", 128, TT, 2, "grec", dt=F32R)
                with nc.allow_low_precision(reason="f32r rounding of softmax denom"):
                    nc.vector.reciprocal(rec_sb[0:1, 0:128], ctxg_ps[64:65, 0:128])
                bc_ps = acc_ps("gbc")
                nc.tensor.matmul(bc_ps[0:64, 0:128], lhsT=ones_all[0:1, 0:64],
                                 rhs=rec_sb[0:1, 0:128], start=True, stop=True)
                bc_sb = sb("tiny", 64, 128, 3, "gbcs")
                nc.vector.tensor_copy(bc_sb[:], bc_ps[0:64, 0:128])
                gst_sb = sb("tiny", 64, 128, 3, "gst")
                nc.vector.tensor_mul(gst_sb[:], ctxg_ps[0:64, 0:128], bc_sb[:])
                nc.sync.dma_start(
                    bass.AP(gc_loc[l], i * 64 * 128, [[128, 64], [1, 128]]),
                    gst_sb[:])
            nc.gpsimd.collective_compute(
                "AllGather", ALU.bypass, ins=[gc_loc[l][:]],
                outs=[gc_all[l][:]], replica_groups=GROUPS)

            # ---- middle sparse attention ----
            ctxT = pool_b.tile([128, DC * NLOC], F32R, tag="big", name="ctxT")
            for hp in range(6):
                q2_sb = q_all[:, hp * NLOC:(hp + 1) * NLOC]
                ik_sb = sb("ikiv", 128, NSLOT, 4, "ik", dt=I32)
                nc.sync.dma_start(ik_sb[:], idx_k[l, hp])
                iv_sb = sb("ikiv", 128, NSLOT, 4, "iv", dt=I32)
                nc.sync.dma_start(iv_sb[:], idx_v[l, hp])
                ks_sb = sb("gath", 128, NSLOT * 65, 3, "ksel", dt=BF16)
                vs_sb = sb("gath", 128, NSLOT * 65, 3, "vsel", dt=BF16)
                nc.vector.memset(
                    vs_sb[:].rearrange("p (j c) -> p j c", c=65)[:, :, 64:65], 1.0)
                # window slots 1..16 are this core's own blocks 0..15 for
                # every core: fetch from kv_loc at static offsets (HWDGE)
                # instead of 32 per-slot SWDGE gathers
                for hh in range(2):
                    h = 2 * hp + hh
                    nc.sync.dma_start(
                        ks_sb[hh * 64:(hh + 1) * 64, 64:17 * 64].rearrange(
                            "p (b t) -> p b t", t=64),
                        bass.AP(kv_loc[l], h * 16 * 4096,
                                [[64, 64], [4096, 16], [1, 64]]))
                    nc.sync.dma_start(
                        vs_sb[hh * 64:(hh + 1) * 64, 65:17 * 65].rearrange(
                            "p (b c) -> p b c", c=65)[:, :, 0:64],
                        bass.AP(kv_loc[l], (192 + h * 16) * 4096,
                                [[64, 64], [4096, 16], [1, 64]]))
                for j in [0] + list(range(17, NSLOT)):
                    nc.gpsimd.indirect_dma_start(
                        out=ks_sb[:, j * 64:(j + 1) * 64],
                        out_offset=None, in_=kv_rows(l),
                        in_offset=bass.IndirectOffsetOnAxis(
                            ap=ik_sb[:, j:j + 1], axis=0))
                    nc.gpsimd.indirect_dma_start(
                        out=vs_sb[:, j * 65:j * 65 + 64],
                        out_offset=None, in_=kv_rows(l),
                        in_offset=bass.IndirectOffsetOnAxis(
                            ap=iv_sb[:, j:j + 1], axis=0))

                if debug and l == 0 and hp == 0:
                    nc.sync.dma_start(
                        dbg["kva"][:].rearrange("(p c) -> p c", p=128),
                        kv_all[l][:].rearrange("(p c) -> p c", p=128))
                    nc.sync.dma_start(dbg["q2"][:], q2_sb)
                    nc.sync.dma_start(dbg["ksel"][:], ks_sb[:, 0:NSLOT * 64])
                    nc.sync.dma_start(dbg["vsel"][:], vs_sb[:])
                ctx_ps = [[pool_ps.tile([65, 512], F32, tag="ctx", bufs=4,
                                        name=f"ctxps{_hh}{_ha}")
                           for _ha in range(2)] for _hh in range(2)]

                # global slots first: they initialize the ctx accumulators
                for g in (18, 19):
                    for half in range(2):
                        sg_ps = acc_ps("sgG")
                        for hh in range(2):
                            pb = hh * 64
                            nc.tensor.matmul(
                                sg_ps[pb:pb + 64, :],
                                lhsT=ks_sb[pb:pb + 64, g * 64:(g + 1) * 64],
                                rhs=q2_sb[pb:pb + 64, half * 512:(half + 1) * 512],
                                start=True, stop=True)
                        sg_sb = sb("sTsb", 128, TT, 2, "sgGsb", dt=BF16)
                        nc.scalar.activation(sg_sb[:], sg_ps[:], AF.Exp,
                                             scale=SCALE)
                        for hh in range(2):
                            pb = hh * 64
                            nc.tensor.matmul(
                                ctx_ps[hh][half][:],
                                lhsT=vs_sb[pb:pb + 64, g * 65:(g + 1) * 65],
                                rhs=sg_sb[pb:pb + 64, :],
                                start=(g == 18), stop=False,
                                skip_group_check=True)
                # window groups
                for grp in WIN_GROUPS:
                    woff = {}
                    off = 0
                    for s in grp:
                        woff[s] = off
                        off += WIN_W[s]
                    sg_ps = acc_ps("sgW")
                    for hh in range(2):
                        pb = hh * 64
                        for s in grp:
                            nc.tensor.matmul(
                                sg_ps[pb:pb + 64, woff[s]:woff[s] + WIN_W[s]],
                                lhsT=ks_sb[pb:pb + 64, s * 64:(s + 1) * 64],
                                rhs=q2_sb[pb:pb + 64,
                                          WIN_QLO[s] * 64:WIN_QLO[s] * 64 + WIN_W[s]],
                                start=True, stop=True)
                    sg_sb = sb("sTsb", 128, TT, 2, "sgWsb", dt=BF16)
                    nc.scalar.activation(sg_sb[:, 0:off], sg_ps[:, 0:off],
                                         AF.Exp, scale=SCALE)
                    for hh in range(2):
                        pb = hh * 64
                        for s in grp:
                            for half in range(2):
                                qa = max(WIN_QLO[s], half * 8)
                                qb = min(WIN_QHI[s], half * 8 + 7)
                                if qa > qb:
                                    continue
                                nc.tensor.matmul(
                                    ctx_ps[hh][half][:, (qa - half * 8) * 64:(qb + 1 - half * 8) * 64],
                                    lhsT=vs_sb[pb:pb + 64, s * 65:(s + 1) * 65],
                                    rhs=sg_sb[pb:pb + 64,
                                              woff[s] + (qa - WIN_QLO[s]) * 64:
                                              woff[s] + (qb + 1 - WIN_QLO[s]) * 64],
                                    start=False, stop=False,
                                    skip_group_check=True)
                # random groups
                for rg in range(6):
                    sg_ps = acc_ps("sgR")
                    for hh in range(2):
                        pb = hh * 64
                        for kk in range(8):
                            k = rg * 8 + kk
                            j = k // 3
                            nc.tensor.matmul(
                                sg_ps[pb:pb + 64, kk * 64:(kk + 1) * 64],
                                lhsT=ks_sb[pb:pb + 64, (20 + k) * 64:(21 + k) * 64],
                                rhs=q2_sb[pb:pb + 64, j * 64:(j + 1) * 64],
                                start=True, stop=True)
                    sg_sb = sb("sTsb", 128, TT, 2, "sgRsb", dt=BF16)
                    nc.scalar.activation(sg_sb[:], sg_ps[:], AF.Exp, scale=SCALE)
                    for hh in range(2):
                        pb = hh * 64
                        for kk in range(8):
                            k = rg * 8 + kk
                            j = k // 3
                            half, jj = j // 8, j % 8
                            nc.tensor.matmul(
                                ctx_ps[hh][half][:, jj * 64:(jj + 1) * 64],
                                lhsT=vs_sb[pb:pb + 64, (20 + k) * 65:(21 + k) * 65],
                                rhs=sg_sb[pb:pb + 64, kk * 64:(kk + 1) * 64],
                                start=False, stop=(k == 47),
                                skip_group_check=True)
                # normalize + place into ctxT
                for hh in range(2):
                    h = 2 * hp + hh
                    for half in range(2):
                        cps = ctx_ps[hh][half]
                        if debug and l == 0 and hp == 0 and hh == 0 and half == 0:
                            cdump = sb("lnt", 128, TT, 6, "cdump")
                            nc.vector.tensor_copy(cdump[0:65, :], cps[:])
                            nc.sync.dma_start(dbg["cps00"][:], cdump[0:65, :])
                        rec_sb = sb("rec", 128, TT, 2, "rec", dt=F32R)
                        with nc.allow_low_precision(reason="f32r rounding of softmax denom"):
                            nc.vector.reciprocal(rec_sb[0:1, :], cps[64:65, :])
                        bc_ps = acc_ps("bcm")
                        nc.tensor.matmul(bc_ps[0:64, :], lhsT=ones_all[0:1, 0:64],
                                         rhs=rec_sb[0:1, :], start=True, stop=True)
                        bc_sb = sb("bcc", 64, TT, 2, "bcs")
                        nc.vector.tensor_copy(bc_sb[:], bc_ps[0:64, :])
                        nc.vector.tensor_mul(
                            ctxT[(h % 2) * 64:(h % 2) * 64 + 64,
                                 (h // 2) * NLOC + half * 512:
                                 (h // 2) * NLOC + (half + 1) * 512],
                            cps[0:64, :], bc_sb[:])

            # ---- blend global pieces ----
            for jj, jsel in ((0, 0), (15, 1)):
                gc_sb = sb("misc", 128, DC * 64, 3, "gcp")
                for h in range(12):
                    r, i = h // 3, h % 3
                    src = bass.AP(gc_all[l],
                                  r * GCTX_ELEMS + i * 64 * 128 + jsel * 64,
                                  [[128, 64], [1, 64]])
                    nc.sync.dma_start(
                        gc_sb[(h % 2) * 64:(h % 2) * 64 + 64,
                              (h // 2) * 64:(h // 2) * 64 + 64], src)
                ctv = ctxT[:].rearrange("p (c t) -> p c t", t=NLOC)[:, :, jj * 64:(jj + 1) * 64]
                tmp_sb = sb("misc", 128, DC * 64, 3, "gblend")
                nc.vector.tensor_scalar_mul(
                    tmp_sb[:].rearrange("p (c t) -> p c t", t=64), ctv,
                    gmask_sb[:, 2 + jsel:3 + jsel])
                nc.vector.scalar_tensor_tensor(
                    ctv, gc_sb[:].rearrange("p (c t) -> p c t", t=64),
                    gmask_sb[:, jsel:jsel + 1],
                    tmp_sb[:].rearrange("p (c t) -> p c t", t=64),
                    ALU.mult, ALU.add)
            # deferred V bias (valid because softmax weights sum to 1)
            for c in range(DC):
                nc.vector.tensor_scalar_add(
                    ctxT[:, c * NLOC:(c + 1) * NLOC],
                    ctxT[:, c * NLOC:(c + 1) * NLOC],
                    bvv_sb[l][:, c:c + 1])
            if debug and l == 0:
                for c in range(DC):
                    nc.sync.dma_start(dbg["ctx0"][:, c, :],
                                      ctxT[:, c * NLOC:(c + 1) * NLOC])

            # ---- Wo + residual + LN1 ----
            wo_t = load_wtiles("wo", lambda c: w_o[l, c], 768)
            aT = pool_b.tile([128, DC * NLOC], F32R, tag="big", name="aT")
            for tt in range(NTT):
                r_sb = sb("stA", 128, DC * TT, 1, "rwo", dt=F32R)

                def wobias(oc, ps, tt=tt, r_sb=r_sb):
                    nc.vector.scalar_tensor_tensor(
                        r_sb[:, oc * TT:(oc + 1) * TT], ps[:],
                        bo_sb[l][:, oc:oc + 1],
                        xT[:, oc * NLOC + tt * TT:oc * NLOC + (tt + 1) * TT],
                        ALU.add, ALU.add)
                proj_pairs(
                    lambda c: wo_t[c][:],
                    lambda c, tt=tt: ctxT[:, c * NLOC + tt * TT:c * NLOC + (tt + 1) * TT],
                    768, wobias)
                layernorm(lambda c: r_sb[:, c * TT:(c + 1) * TT],
                          ln1g_sb[l], ln1b_sb[l],
                          lambda c: aT[:, c * NLOC + tt * TT:c * NLOC + (tt + 1) * TT])
            if debug and l == 0:
                for c in range(DC):
                    nc.sync.dma_start(dbg["a0"][:, c, :],
                                      aT[:, c * NLOC:(c + 1) * NLOC])

            # ---- FFN + residual + LN2 ----
            xT_next = pool_b.tile([128, DC * NLOC], F32R, tag="big",
                                  name="xT_next")
            for tt in range(NTT):
                r2_sb = sb("stA", 128, DC * TT, 1, "rffn", dt=F32R)
                accs = ([pool_ps.tile([128, TT], F32, tag="acc", bufs=4,
                                      name=f"dpsa{oc}") for oc in range(3)] +
                        [pool_ps.tile([128, TT], F32, tag="ctx", bufs=4,
                                      name=f"dpsc{oc}") for oc in range(3)])
                for fc in range(FFC):
                    hps = pool_ps.tile([128, TT], F32, tag="ctx", bufs=4,
                                       name="hps")
                    wi_t = sb("wistr", 128, 768, 2, "wit", dt=F32R)
                    nc.sync.dma_start(
                        wi_t[:].rearrange("p (c k) -> p c k", k=128),
                        w_i[l, :, :, fc * 128:(fc + 1) * 128].rearrange(
                            "c p k -> p c k"))
                    for c in range(DC):
                        nc.tensor.matmul(
                            hps[:], lhsT=(wi_t[:, c * 128:(c + 1) * 128]),
                            rhs=(aT[:, c * NLOC + tt * TT:c * NLOC + (tt + 1) * TT]),
                            start=(c == 0), stop=(c == DC - 1))
                    hpr_sb = sb("gel", 128, TT, 4, "ghp", dt=F32R)
                    nc.scalar.activation(hpr_sb[:], hps[:], AF.Gelu_apprx_tanh,
                                         bias=bi_sb[l][:, fc:fc + 1])
                    wd_t = sb("wdstr", 128, 768, 2, "wdt", dt=F32R)
                    nc.sync.dma_start(wd_t[:], w_d[l, fc])
                    for oc in range(DC):
                        nc.tensor.matmul(
                            accs[oc][:], lhsT=(wd_t[:, oc * 128:(oc + 1) * 128]),
                            rhs=(hpr_sb[:]),
                            start=(fc == 0), stop=(fc == FFC - 1),
                            skip_group_check=True)
                for oc in range(DC):
                    nc.vector.scalar_tensor_tensor(
                        r2_sb[:, oc * TT:(oc + 1) * TT], accs[oc][:],
                        bd_sb[l][:, oc:oc + 1],
                        aT[:, oc * NLOC + tt * TT:oc * NLOC + (tt + 1) * TT],
                        ALU.add, ALU.add)
                layernorm(lambda c: r2_sb[:, c * TT:(c + 1) * TT],
                          ln2g_sb[l], ln2b_sb[l],
                          lambda c: xT_next[:, c * NLOC + tt * TT:c * NLOC + (tt + 1) * TT])
            xT = xT_next
            if debug and l == 0:
                for c in range(DC):
                    nc.sync.dma_start(dbg["x1"][:, c, :],
                                      xT[:, c * NLOC:(c + 1) * NLOC])

        # ---------- pooled mean + fc ----------
        xsum_sb = sb("tiny", 128, DC, 3, "xsum")
        for c in range(DC):
            nc.vector.reduce_sum(xsum_sb[:, c:c + 1],
                                 xT[:, c * NLOC:(c + 1) * NLOC],
                                 axis=mybir.AxisListType.X)
        fc_ps = ctx_ps_t("fcps")
        for c in range(DC):
            nc.tensor.matmul(fc_ps[0:1, 0:1], lhsT=xsum_sb[:, c:c + 1],
                             rhs=fcw_sb[:, c:c + 1],
                             start=(c == 0), stop=(c == DC - 1))
        ofc_sb = sb("tiny2", 1, 1, 2, "ofc")
        nc.vector.tensor_copy(ofc_sb[:], fc_ps[0:1, 0:1])
        nc.sync.dma_start(out_fc[:], ofc_sb[:])

    nc.compile()
    return nc


# ======================= host side =======================

def _vec128(v):
    return np.ascontiguousarray(np.asarray(v, np.float32).reshape(DC, 128).T)


def _slot_k(g, h):
    return (g // 16) * SLOTS_RANK + h * 16 + (g % 16)


def _slot_v(g, h):
    return (g // 16) * SLOTS_RANK + 192 + h * 16 + (g % 16)


def _make_gindices(c):
    igk = np.zeros((L, 3, 64, 64), np.int32)
    igv = np.zeros((L, 3, 64, 64), np.int32)
    p = np.arange(64)
    for l in range(L):
        for i in range(3):
            h = 3 * c + i
            sk = np.array([_slot_k(g, h) for g in range(64)])
            sv = np.array([_slot_v(g, h) for g in range(64)])
            igk[l, i] = sk[None, :] * 64 + p[:, None]
            igv[l, i] = sv[None, :] * 64 + p[:, None]
    return igk, igv


def _make_indices(rand_blocks, c):
    ik = np.zeros((L, 6, 128, NSLOT), np.int32)
    iv = np.zeros((L, 6, 128, NSLOT), np.int32)
    p = np.arange(64)
    for l in range(L):
        for hp in range(6):
            for hh in range(2):
                h = 2 * hp + hh
                gs = np.zeros(NSLOT, np.int64)
                for s in range(18):
                    gs[s] = min(max(c * 16 - 1 + s, 0), 63)
                gs[18], gs[19] = 0, 63
                for j in range(16):
                    m = c * 16 + j
                    for r_ in range(R):
                        gs[20 + j * 3 + r_] = (rand_blocks[l, h, m, r_]
                                               if 1 <= m <= 62 else 0)
                sk = np.array([_slot_k(g, h) for g in gs])
                sv = np.array([_slot_v(g, h) for g in gs])
                ik[l, hp, hh * 64:(hh + 1) * 64, :] = sk[None, :] * 64 + p[:, None]
                iv[l, hp, hh * 64:(hh + 1) * 64, :] = sv[None, :] * 64 + p[:, None]
    return ik, iv


_CACHE = {}


def _get_nc(debug=False):
    key = "dbg" if debug else "plain"
    if key not in _CACHE:
        _CACHE[key] = build(debug)
    return _CACHE[key]


def make_in_maps(inputs):
    f32 = lambda a: np.ascontiguousarray(np.asarray(a, np.float32))
    inp = {k: np.asarray(v) for k, v in inputs.items()}
    w_qkv = f32(np.concatenate([inp["Wq"], inp["Wk"], inp["Wv"]], axis=2)
                .reshape(L, DC, 128, 2304))
    w_o = f32(inp["Wo"].reshape(L, DC, 128, D))
    w_i = f32(inp["Wi"].reshape(L, DC, 128, FF))
    w_d = f32(np.asarray(inp["Wd"], np.float32).reshape(L, FFC, 128, D))
    b_qk = f32(np.stack([np.concatenate(
        [_vec128(inp["bq"][l]), _vec128(inp["bk"][l])], axis=1)
        for l in range(L)]))
    b_v = f32(np.stack([_vec128(inp["bv"][l]) for l in range(L)]))
    b_o = f32(np.stack([_vec128(inp["bo"][l]) for l in range(L)]))
    b_i = f32(np.stack([np.ascontiguousarray(
        np.asarray(inp["bi"][l], np.float32).reshape(FFC, 128).T)
        for l in range(L)]))
    b_d = f32(np.stack([_vec128(inp["bd"][l]) for l in range(L)]))
    emb_g, emb_b = _vec128(inp["emb_ln_g"]), _vec128(inp["emb_ln_b"])
    ln1_g = f32(np.stack([_vec128(inp["ln1_g"][l]) for l in range(L)]))
    ln1_b = f32(np.stack([_vec128(inp["ln1_b"][l]) for l in range(L)]))
    ln2_g = f32(np.stack([_vec128(inp["ln2_g"][l]) for l in range(L)]))
    ln2_b = f32(np.stack([_vec128(inp["ln2_b"][l]) for l in range(L)]))
    fc_w = _vec128(inp["fc_w"][:, 0])
    pos_tt = f32(inp["pos_emb"] + np.asarray(inp["tt_emb"])[None, :])
    emb = f32(inp["inputs_embeds"])
    rand_blocks = np.asarray(inp["rand_blocks"])

    idx_cache = {}
    in_maps = []
    for core in range(8):
        b, c = core // 4, core % 4
        rows = slice(c * NLOC, (c + 1) * NLOC)
        xT0 = np.ascontiguousarray(
            emb[b, rows].T.reshape(DC, 128, NLOC).transpose(1, 0, 2))
        posTa = np.ascontiguousarray(
            pos_tt[rows].T.reshape(DC, 128, NLOC).transpose(1, 0, 2))
        if c not in idx_cache:
            idx_cache[c] = _make_indices(rand_blocks, c) + _make_gindices(c)
        ik, iv, igk, igv = idx_cache[c]
        m0, m15 = float(c == 0), float(c == 3)
        gm = np.zeros((128, 4), np.float32)
        gm[:, 0], gm[:, 1], gm[:, 2], gm[:, 3] = m0, m15, 1 - m0, 1 - m15
        in_maps.append({
            "xT0": xT0, "posT": posTa, "w_qkv": w_qkv, "w_o": w_o,
            "w_i": w_i, "w_d": w_d, "b_qk": b_qk, "b_v": b_v,
            "b_o": b_o, "b_i": b_i, "b_d": b_d, "emb_g": emb_g,
            "emb_b": emb_b, "ln1_g": ln1_g, "ln1_b": ln1_b, "ln2_g": ln2_g,
            "ln2_b": ln2_b, "fc_w": fc_w, "idx_k": ik, "idx_v": iv,
            "gmask": gm, "hbase": np.array([[3 * c]], np.uint32),
            "ones_in": np.ones((128, 128), np.float32),
            "idx_gk": igk, "idx_gv": igv,
        })
    return in_maps


def finish(inputs, results):
    fc_b = float(np.asarray(inputs["fc_b"])[0])
    out = np.zeros(B, np.float32)
    for b in range(B):
        tot = sum(float(results[4 * b + c]["out_fc"][0, 0]) for c in range(4))
        out[b] = tot / N + fc_b
    return out.astype(np.float32)


class _Runner:
    """Cached shard_map-jitted executable over the 8 NeuronCores.

    Mirrors bass2jax.run_bass_via_pjrt's multi-core path, but caches the
    jitted callable so repeated kernel() calls don't retrace/recompile, and
    exposes device-resident-input execution for timing.
    """

    def __init__(self, nc):
        import jax
        import concourse.mybir as mybir_
        from concourse import bass2jax
        from jax.sharding import Mesh, PartitionSpec, NamedSharding
        bass2jax.install_neuronx_cc_hook()
        self.jax = jax
        in_names, out_names, out_avals = [], [], []
        pname = nc.partition_id_tensor.name if nc.partition_id_tensor else None
        for alloc in nc.m.functions[0].allocations:
            if not isinstance(alloc, mybir_.MemoryLocationSet):
                continue
            name = alloc.memorylocations[0].name
            if alloc.kind == "ExternalInput":
                if name != pname:
                    in_names.append(name)
            elif alloc.kind == "ExternalOutput":
                out_names.append(name)
                out_avals.append(jax.core.ShapedArray(
                    tuple(alloc.tensor_shape), mybir_.dt.np(alloc.dtype)))
        self.in_names, self.out_names, self.out_avals = in_names, out_names, out_avals
        n_params, n_outs = len(in_names), len(out_avals)
        all_names = in_names + out_names
        if pname is not None:
            all_names.append(pname)

        def _body(*args):
            operands = list(args)
            if pname is not None:
                operands.append(bass2jax.partition_id_tensor())
            outs = bass2jax._bass_exec_p.bind(
                *operands, out_avals=tuple(out_avals),
                in_names=tuple(all_names), out_names=tuple(out_names),
                lowering_input_output_aliases=(),
                sim_require_finite=True, sim_require_nnan=True, nc=nc)
            return tuple(outs)

        from jax.experimental.shard_map import shard_map
        devices = jax.devices()[:8]
        self.mesh = Mesh(np.asarray(devices), ("core",))
        in_specs = (PartitionSpec("core"),) * (n_params + n_outs)
        out_specs = (PartitionSpec("core"),) * n_outs
        self.sharding = NamedSharding(self.mesh, PartitionSpec("core"))
        self.fn = jax.jit(shard_map(_body, mesh=self.mesh, in_specs=in_specs,
                                    out_specs=out_specs, check_rep=False),
                          keep_unused=True)
        self.n_params, self.n_outs = n_params, n_outs

    def device_args(self, in_maps):
        jax = self.jax
        concat_in = [np.concatenate([np.asarray(in_maps[c][n])
                                     for c in range(8)], axis=0)
                     for n in self.in_names]
        concat_zero = [np.zeros((8 * a.shape[0], *a.shape[1:]), a.dtype)
                       for a in self.out_avals]
        return [jax.device_put(a, self.sharding)
                for a in concat_in + concat_zero]

    def run_device(self, dargs):
        outs = self.fn(*dargs)
        self.jax.block_until_ready(outs)
        return outs

    def run(self, in_maps):
        outs = self.run_device(self.device_args(in_maps))
        res = []
        for c in range(8):
            res.append({n: np.asarray(outs[i]).reshape(
                8, *self.out_avals[i].shape)[c]
                for i, n in enumerate(self.out_names)})
        return res


def _get_runner():
    if "runner" not in _CACHE:
        _CACHE["runner"] = _Runner(_get_nc())
    return _CACHE["runner"]


def kernel(**inputs):
    runner = _get_runner()
    in_maps = make_in_maps(inputs)
    return finish(inputs, runner.run(in_maps))



# revision 42
# speedup vs baseline: 1.3707x; 1.3707x over previous
"""BigBird regressor forward pass on 8 Trainium2 NeuronCores (Bass/Tile).

Sharding: 8 cores = batch(2) x sequence-chunks(4). Each core owns 1024 tokens
of one batch in transposed layout x^T [768, 1024]. Per layer:
  - K/V projections local, written to DRAM block-major in bf16; all Q
    projections computed up front (hidden behind the AllGather)
  - AllGather K,V (+x edge blocks) within each 4-core group
  - full attention for global blocks 0/63 split by heads across the group,
    exchanged with a small second AllGather
  - middle-block sparse attention fully local and index-driven: the host
    precomputes row-gather indices (window/global/random block slots) so every
    matmul access pattern is static; scores are computed transposed (keys on
    PSUM partitions) so no transposes are needed; V tiles carry an appended
    ones column so softmax denominators fall out of the same matmuls
  - Wo, FFN, LayerNorms are sequence-local (no more collectives).
Output: per-core scalar partial of sum_tokens(x_final . fc_w); host reduces.

Precision/perf choices (tolerance is rel 2e-2; measured ~2e-4):
  - dense projections run as float32r (1 PE cycle/col vs 4 for fp32); every
    producer tile feeding an f32r matmul is declared float32r per the BIR
    verifier's rounding rule
  - attention K/V/Q and gathered tiles are bf16 (halves AllGather + gather
    bytes); PSUM accumulation stays fp32
  - phase weights (K/V/Q/Wo) are SBUF-resident per layer, loaded with a few
    wide DMAs; FFN weights stream one [128,768] tile per fc chunk
  - GELU is a single fused Gelu_apprx_tanh activation with per-partition bias
  - K bias is dropped (constant shift per query under softmax); V bias is
    deferred to one per-partition add on the attention output
  - softmax-denominator broadcasts use K=1 matmuls against a host-loaded ones
    tile (f32r memset is invalid ISA)
Note: indirect row gathers must stay one-index-per-partition ([128,1] offset
APs); batched [128,k] offsets execute in CoreSim but corrupt data on HW.
"""
import contextlib

import numpy as np

import concourse.bass as bass
import concourse.bacc as bacc
import concourse.mybir as mybir
import concourse.tile as tile
from concourse.bass import ds

F32 = mybir.dt.float32
F32R = mybir.dt.float32r
BF16 = mybir.dt.bfloat16
I32 = mybir.dt.int32
U32 = mybir.dt.uint32
AF = mybir.ActivationFunctionType
ALU = mybir.AluOpType




B, N, D, H, L = 2, 4096, 768, 12, 2
DH, BS, NB, R, FF = 64, 64, 64, 3, 3072
DC = D // 128
FFC = FF // 128
NLOC = N // 4
TT = 512
NTT = NLOC // TT
EPS = 1e-12
SCALE = 0.125
GELU_C = float(np.sqrt(2.0 / np.pi))
GELU_A = 0.044715

SLOTS_RANK = 408
KV_ELEMS = SLOTS_RANK * 4096
EDGE_OFF = 384 * 4096
GCTX_ELEMS = 3 * 64 * 128

NSLOT = 68
WIN_W = [64, 128] + [192] * 14 + [128, 64]
WIN_QLO = [max(s - 2, 0) for s in range(18)]
WIN_QHI = [min(s, 15) for s in range(18)]
WIN_GROUPS = [[0, 1, 2], [3, 4], [5, 6], [7, 8], [9, 10], [11, 12], [13, 14],
              [15, 16, 17]]
GROUPS = [[0, 1, 2, 3], [4, 5, 6, 7]]


def build(debug=False):
    nc = bacc.Bacc("TRN2", target_bir_lowering=False, debug=False,
                   num_devices=8)

    xT0 = nc.dram_tensor("xT0", [128, DC, NLOC], F32, kind="ExternalInput")
    posT = nc.dram_tensor("posT", [128, DC, NLOC], F32, kind="ExternalInput")
    w_qkv = nc.dram_tensor("w_qkv", [L, DC, 128, 2304], F32R, kind="ExternalInput")
    w_o = nc.dram_tensor("w_o", [L, DC, 128, D], F32R, kind="ExternalInput")
    w_i = nc.dram_tensor("w_i", [L, DC, 128, FF], F32R, kind="ExternalInput")
    w_d = nc.dram_tensor("w_d", [L, FFC, 128, D], F32R, kind="ExternalInput")
    b_qk = nc.dram_tensor("b_qk", [L, 128, 12], F32, kind="ExternalInput")
    b_v = nc.dram_tensor("b_v", [L, 128, DC], F32, kind="ExternalInput")
    b_o = nc.dram_tensor("b_o", [L, 128, DC], F32, kind="ExternalInput")
    b_i = nc.dram_tensor("b_i", [L, 128, FFC], F32, kind="ExternalInput")
    b_d = nc.dram_tensor("b_d", [L, 128, DC], F32, kind="ExternalInput")
    emb_g = nc.dram_tensor("emb_g", [128, DC], F32, kind="ExternalInput")
    emb_b = nc.dram_tensor("emb_b", [128, DC], F32, kind="ExternalInput")
    ln1_g = nc.dram_tensor("ln1_g", [L, 128, DC], F32, kind="ExternalInput")
    ln1_b = nc.dram_tensor("ln1_b", [L, 128, DC], F32, kind="ExternalInput")
    ln2_g = nc.dram_tensor("ln2_g", [L, 128, DC], F32, kind="ExternalInput")
    ln2_b = nc.dram_tensor("ln2_b", [L, 128, DC], F32, kind="ExternalInput")
    fc_w = nc.dram_tensor("fc_w", [128, DC], F32, kind="ExternalInput")
    idx_k = nc.dram_tensor("idx_k", [L, 6, 128, NSLOT], I32, kind="ExternalInput")
    idx_v = nc.dram_tensor("idx_v", [L, 6, 128, NSLOT], I32, kind="ExternalInput")
    gmask = nc.dram_tensor("gmask", [128, 4], F32, kind="ExternalInput")
    idx_gk = nc.dram_tensor("idx_gk", [L, 3, 64, 64], I32, kind="ExternalInput")
    idx_gv = nc.dram_tensor("idx_gv", [L, 3, 64, 64], I32, kind="ExternalInput")
    hbase = nc.dram_tensor("hbase", [1, 1], U32, kind="ExternalInput")
    ones_in = nc.dram_tensor("ones_in", [128, 128], F32R, kind="ExternalInput")
    out_fc = nc.dram_tensor("out_fc", [1, 1], F32, kind="ExternalOutput")

    dbg = {}
    if debug:
        for nm in ("xln", "x1", "ctx0", "a0"):
            dbg[nm] = nc.dram_tensor("dbg_" + nm, [128, DC, NLOC], F32R,
                                     kind="ExternalOutput")
        dbg["q2"] = nc.dram_tensor("dbg_q2", [128, NLOC], BF16,
                                   kind="ExternalOutput")
        dbg["ksel"] = nc.dram_tensor("dbg_ksel", [128, NSLOT * 64], BF16,
                                     kind="ExternalOutput")
        dbg["vsel"] = nc.dram_tensor("dbg_vsel", [128, NSLOT * 65], BF16,
                                     kind="ExternalOutput")
        dbg["cps00"] = nc.dram_tensor("dbg_cps00", [65, 512], F32,
                                      kind="ExternalOutput")
        dbg["kva"] = nc.dram_tensor("dbg_kva", [4 * KV_ELEMS], BF16,
                                    kind="ExternalOutput")

    kv_loc = [nc.dram_tensor(f"kv_loc{l}", [KV_ELEMS], BF16) for l in range(L)]
    kv_all = [nc.dram_tensor(f"kv_all{l}", [4 * KV_ELEMS], BF16) for l in range(L)]
    gc_loc = [nc.dram_tensor(f"gc_loc{l}", [GCTX_ELEMS], F32) for l in range(L)]
    gc_all = [nc.dram_tensor(f"gc_all{l}", [4 * GCTX_ELEMS], F32) for l in range(L)]

    with tile.TileContext(nc) as tc, contextlib.ExitStack() as ex:
        pool_c = ex.enter_context(tc.tile_pool(name="consts", bufs=1))
        pool_b = ex.enter_context(tc.tile_pool(name="bigp", bufs=3))
        pool = ex.enter_context(tc.tile_pool(name="gen", bufs=1))
        pool_ps = ex.enter_context(tc.tile_pool(name="psum", bufs=4, space="PSUM"))

        def acc_ps(name):
            return pool_ps.tile([128, TT], F32, tag="acc", name=name, bufs=4)

        def ctx_ps_t(name):
            return pool_ps.tile([128, TT], F32, tag="ctx", name=name, bufs=4)

        def sb(tag, p, f, bufs, name, dt=F32):
            return pool.tile([p, f], dt, tag=tag, name=name, bufs=bufs)

        # ---------- constants ----------
        ones128 = pool_c.tile([128, 1], F32, tag="c_o128", name="ones128")
        nc.vector.memset(ones128[:], 1.0)
        ones_all = pool_c.tile([128, 128], F32R, tag="c_oall", name="ones_all")
        nc.sync.dma_start(ones_all[:], ones_in[:])
        eps_sb = pool_c.tile([128, 1], F32, tag="c_eps", name="epsc")
        nc.vector.memset(eps_sb[:], EPS)

        def ldconst(name, src, dt=F32):
            tl = pool_c.tile(list(src.shape), dt, tag=f"c_{name}", name=name)
            nc.sync.dma_start(tl[:], src[:])
            return tl

        bqk_sb = [ldconst(f"bqk{l}", b_qk[l]) for l in range(L)]
        bo_sb = [ldconst(f"bo{l}", b_o[l]) for l in range(L)]
        bi_sb = [ldconst(f"bi{l}", b_i[l]) for l in range(L)]
        bd_sb = [ldconst(f"bd{l}", b_d[l]) for l in range(L)]
        embg_sb = ldconst("embg", emb_g)
        embb_sb = ldconst("embb", emb_b)
        ln1g_sb = [ldconst(f"ln1g{l}", ln1_g[l]) for l in range(L)]
        ln1b_sb = [ldconst(f"ln1b{l}", ln1_b[l]) for l in range(L)]
        ln2g_sb = [ldconst(f"ln2g{l}", ln2_g[l]) for l in range(L)]
        ln2b_sb = [ldconst(f"ln2b{l}", ln2_b[l]) for l in range(L)]
        fcw_sb = ldconst("fcw", fc_w)
        gmask_sb = ldconst("gmask", gmask)

        bvv_sb = [ldconst(f"bvv{l}", b_v[l]) for l in range(L)]

        rv_dve = nc.vector.alloc_register("hb_dve")
        nc.vector.reg_load(rv_dve, hbase[0:1, 0:1])
        sv_dve = nc.snap(rv_dve, min_val=0, max_val=9)

        def kv_rows(l):
            return kv_all[l][:].rearrange("(r c) -> r c", c=64)

        def kv_slots(l):
            return kv_all[l][:].rearrange("(s p t) -> s p t", p=64, t=64)

        # ---------- LayerNorm (x^T layout: stats over partitions via PE) ----
        def layernorm(r_get, g_sb, b_sb, out_get):
            # stats matmuls use an all-ones [128,128] lhsT so every output
            # partition receives the column sum (pre-broadcast, no K=1 mm)
            mu_ps = pool_ps.tile([128, TT], F32, tag="ctx", name="lnmu_ps", bufs=4)
            sq_ps = pool_ps.tile([128, TT], F32, tag="ctx", name="lnsq_ps", bufs=4)
            for c in range(DC):
                nc.tensor.matmul(mu_ps[:], lhsT=(ones_all[:]), rhs=(r_get(c)),
                                 start=(c == 0), stop=(c == DC - 1))
            for c in range(DC):
                sq_sb = sb("lnt", 128, TT, 6, "lnsq", dt=F32R)
                nc.scalar.square(sq_sb[:], r_get(c))
                nc.tensor.matmul(sq_ps[:], lhsT=(ones_all[:]), rhs=(sq_sb[:]),
                                 start=(c == 0), stop=(c == DC - 1))
            mu_sb = sb("lnt", 128, TT, 6, "lnmu")
            nc.vector.tensor_scalar_mul(mu_sb[:], mu_ps[:], 1.0 / D)
            var_sb = sb("lnt", 128, TT, 6, "lnvar")
            nc.vector.tensor_scalar_mul(var_sb[:], sq_ps[:], 1.0 / D)
            mu2_sb = sb("lnt", 128, TT, 6, "lnmu2")
            nc.vector.tensor_mul(mu2_sb[:], mu_sb[:], mu_sb[:])
            nc.vector.tensor_sub(var_sb[:], var_sb[:], mu2_sb[:])
            std_sb = sb("lnt", 128, TT, 6, "lnstd")
            nc.scalar.activation(std_sb[:], var_sb[:], AF.Sqrt,
                                 bias=eps_sb[:, 0:1])
            rinv_sb = sb("lnt", 128, TT, 6, "lnrinv")
            nc.vector.reciprocal(rinv_sb[:], std_sb[:])
            for c in range(DC):
                u_sb = sb("lnt", 128, TT, 6, "lnu")
                nc.vector.tensor_sub(u_sb[:], r_get(c), mu_sb[:])
                v_sb = sb("lnt", 128, TT, 6, "lnv")
                nc.vector.scalar_tensor_tensor(v_sb[:], u_sb[:],
                                               g_sb[:, c:c + 1], rinv_sb[:],
                                               ALU.mult, ALU.mult)
                nc.vector.tensor_scalar_add(out_get(c), v_sb[:],
                                            b_sb[:, c:c + 1])

        # dense projection helper: output chunks in pairs, weights from
        # SBUF-resident phase tiles (wget(c) -> [128, ncol] AP)
        def proj_pairs(wget, rhs_get, ncol, bias_apply, nK=DC, rw=TT):
            npc = ncol // 128
            for pg in range(0, npc, 2):
                w2 = min(2, npc - pg)
                pss = [acc_ps(f"prj{pg + j}") for j in range(w2)]
                for c in range(nK):
                    wt = wget(c)
                    for j in range(w2):
                        nc.tensor.matmul(
                            pss[j][:, 0:rw],
                            lhsT=(wt[:, (pg + j) * 128:(pg + j + 1) * 128]),
                            rhs=(rhs_get(c)), start=(c == 0),
                            stop=(c == nK - 1))
                for j in range(w2):
                    bias_apply(pg + j, pss[j])

        # load one phase's worth of weights (6 chunks) into the shared
        # "wph" rotation; phases are sequential so 6 bufs suffice
        def load_wtiles(nm, src_get, ncols, n=DC):
            tiles = []
            for c in range(n):
                t = pool.tile([128, ncols], F32R, tag="wph", name=f"{nm}{c}",
                              bufs=DC)
                nc.sync.dma_start(t[:], src_get(c))
                tiles.append(t)
            return tiles

        # ---------- embeddings + LN ----------
        xT = pool_b.tile([128, DC * NLOC], F32R, tag="big", name="xT_emb")
        for tt in range(NTT):
            r_sb = sb("stA", 128, DC * TT, 1, "embr", dt=F32R)
            for c in range(DC):
                x0_sb = sb("gel", 128, TT, 4, "embx")
                nc.sync.dma_start(x0_sb[:], xT0[:, c, tt * TT:(tt + 1) * TT])
                p0_sb = sb("gel", 128, TT, 4, "embp")
                nc.sync.dma_start(p0_sb[:], posT[:, c, tt * TT:(tt + 1) * TT])
                nc.vector.tensor_add(r_sb[:, c * TT:(c + 1) * TT], x0_sb[:],
                                     p0_sb[:])
            layernorm(lambda c: r_sb[:, c * TT:(c + 1) * TT],
                      embg_sb, embb_sb,
                      lambda c: xT[:, c * NLOC + tt * TT:c * NLOC + (tt + 1) * TT])
        if debug:
            for c in range(DC):
                nc.sync.dma_start(dbg["xln"][:, c, :],
                                  xT[:, c * NLOC:(c + 1) * NLOC])

        # ---------- layers ----------
        for l in range(L):
            # ---- K projection -> kv_loc (block-major [d, t]) ----
            wk = load_wtiles("wk", lambda c: w_qkv[l, c, :, 768:1536], 768)
            for tt in range(NTT):
                def kbias(kc, ps, tt=tt):
                    st = sb("gel", 128, TT, 4, "kstage", dt=BF16)
                    nc.vector.tensor_scalar_add(st[:], ps[:],
                                                bqk_sb[l][:, 6 + kc:7 + kc])
                    for hh in range(2):
                        h = 2 * kc + hh
                        dst = bass.AP(kv_loc[l], (h * 16 + tt * 8) * 4096,
                                      [[64, 64], [4096, 8], [1, 64]])
                        nc.sync.dma_start(
                            dst, st[hh * 64:(hh + 1) * 64, :].rearrange(
                                "p (b t) -> p b t", t=64))
                proj_pairs(
                    lambda c: wk[c][:],
                    lambda c, tt=tt: xT[:, c * NLOC + tt * TT:c * NLOC + (tt + 1) * TT],
                    768, kbias)
            # ---- V projection (natural layout [tok, dd]) -> kv_loc ----
            wv = load_wtiles("wv", lambda c: w_qkv[l, c, :, 1536:2304], 768)
            for tcb in range(8):
                vst = sb("vstage", 128, 768, 2, "vstage", dt=BF16)
                for half in range(2):
                    vps = acc_ps(f"vps{half}")
                    for c in range(DC):
                        nc.tensor.matmul(
                            vps[:, 0:384],
                            lhsT=(xT[:, c * NLOC + tcb * 128:c * NLOC + (tcb + 1) * 128]),
                            rhs=(wv[c][:, half * 384:(half + 1) * 384]),
                            start=(c == 0),
                            stop=(c == DC - 1), skip_group_check=True)
                    # V stored WITHOUT bias: since softmax rows sum to 1,
                    # bv is added once to ctxT after attention instead
                    nc.vector.tensor_copy(vst[:, half * 384:(half + 1) * 384],
                                          vps[:, 0:384])
                for hh in range(2):
                    bt = 2 * tcb + hh
                    dst = bass.AP(kv_loc[l], (192 + bt) * 4096,
                                  [[64, 64], [16 * 4096, 12], [1, 64]])
                    nc.sync.dma_start(
                        dst, vst[hh * 64:(hh + 1) * 64, :].rearrange(
                            "p (h d) -> p h d", d=64))
            # ---- x edges -> kv_loc ----
            for eb in range(2):
                t0 = 0 if eb == 0 else NLOC - 64
                src = xT[:].rearrange("p (c t) -> p c t", t=NLOC)[:, :, t0:t0 + 64]
                xe_b = sb("xeb", 128, DC * 128, 2, "xeb", dt=BF16)
                nc.vector.tensor_copy(
                    xe_b[:, 0:DC * 64].rearrange("p (c t) -> p c t", t=64), src)
                dst = bass.AP(kv_loc[l], EDGE_OFF + eb * (DC * 128 * 64),
                              [[64, 128], [128 * 64, DC], [1, 64]])
                nc.sync.dma_start(
                    dst, xe_b[:, 0:DC * 64].rearrange("p (c t) -> p c t", t=64))

            # Q weights + all six head-pair Q projections run BEFORE the
            # AllGather is awaited, hiding the collective behind PE work
            wq = load_wtiles("wq", lambda c: w_qkv[l, c, :, 0:768], 768)
            q_all = sb("qall", 128, 6 * NLOC, 1, "qall", dt=BF16)
            nc.gpsimd.collective_compute(
                "AllGather", ALU.bypass, ins=[kv_loc[l][:]],
                outs=[kv_all[l][:]], replica_groups=GROUPS)
            for hp in range(6):
                for tt in range(NTT):
                    def qbias(_pg, ps, hp=hp, tt=tt):
                        nc.vector.tensor_scalar_add(
                            q_all[:, hp * NLOC + tt * TT:hp * NLOC + (tt + 1) * TT],
                            ps[:], bqk_sb[l][:, hp:hp + 1])
                    proj_pairs(
                        lambda c: wq[c][:, hp * 128:(hp + 1) * 128],
                        lambda c, tt=tt: xT[:, c * NLOC + tt * TT:c * NLOC + (tt + 1) * TT],
                        128, qbias)

            # ---- global-block (full) attention ----
            xg_b = sb("xeb", 128, DC * 128, 2, "xgb", dt=BF16)
            for eb in range(2):
                rbase = (0 if eb == 0 else 3) * KV_ELEMS
                src = bass.AP(kv_all[l], rbase + EDGE_OFF + eb * (DC * 128 * 64),
                              [[64, 128], [128 * 64, DC], [1, 64]])
                nc.sync.dma_start(
                    xg_b[:].rearrange(
                        "p (c t) -> p c t", t=128)[:, :, eb * 64:(eb + 1) * 64],
                    src)
            xg_sb = sb("stA", 128, DC * TT, 1, "xg", dt=F32R)
            nc.vector.tensor_copy(xg_sb[:, 0:DC * 128], xg_b[:])
            qg_sb = sb("qbuf", 64, 12 * 128, 1, "qg", dt=BF16)

            def qgbias(oc, ps):
                for hh in range(2):
                    h = 2 * oc + hh
                    nc.vector.tensor_scalar_add(
                        qg_sb[:, h * 128:(h + 1) * 128],
                        ps[hh * 64:(hh + 1) * 64, 0:128],
                        bqk_sb[l][hh * 64:(hh + 1) * 64, oc:oc + 1])
            proj_pairs(
                lambda c: wq[c][:],
                lambda c: xg_sb[:, c * 128:(c + 1) * 128],
                768, qgbias, rw=128)

            for i in range(3):
                kf_sb = sb("gath", 128, NSLOT * 65, 3, "kf", dt=BF16)
                va_sb = sb("gath", 128, NSLOT * 65, 3, "va", dt=BF16)
                kfull = kf_sb[0:64, 0:4096]
                vaug = va_sb[0:64, 0:64 * 65]
                igk_sb = sb("ikiv", 128, NSLOT, 4, "igk", dt=I32)
                nc.sync.dma_start(igk_sb[0:64, 0:64], idx_gk[l, i])
                igv_sb = sb("ikiv", 128, NSLOT, 4, "igv", dt=I32)
                nc.sync.dma_start(igv_sb[0:64, 0:64], idx_gv[l, i])
                for g in range(64):
                    nc.gpsimd.indirect_dma_start(
                        out=kfull[:, g * 64:(g + 1) * 64],
                        out_offset=None, in_=kv_rows(l),
                        in_offset=bass.IndirectOffsetOnAxis(
                            ap=igk_sb[0:64, g:g + 1], axis=0))
                    nc.gpsimd.indirect_dma_start(
                        out=vaug[:, g * 65:g * 65 + 64],
                        out_offset=None, in_=kv_rows(l),
                        in_offset=bass.IndirectOffsetOnAxis(
                            ap=igv_sb[0:64, g:g + 1], axis=0))
                nc.vector.memset(
                    vaug.rearrange("p (g c) -> p g c", c=65)[:, :, 64:65], 1.0)
                qgi_sb = sb("tiny", 64, 128, 3, "qgi", dt=BF16)
                nc.vector.tensor_copy(qgi_sb[:],
                                      qg_sb[:, ds((sv_dve + i) * 128, 128)])
                ctxg_ps = ctx_ps_t("ctxg")
                for gg in range(16):
                    sg_ps = acc_ps("sgF")
                    for k in range(4):
                        g = gg * 4 + k
                        nc.tensor.matmul(sg_ps[0:64, k * 128:(k + 1) * 128],
                                         lhsT=kfull[:, g * 64:(g + 1) * 64],
                                         rhs=qgi_sb[:], start=True, stop=True)
                    sg_sb = sb("sTsb", 128, TT, 2, "sgFsb", dt=BF16)
                    nc.scalar.activation(sg_sb[0:64, :], sg_ps[0:64, :],
                                         AF.Exp, scale=SCALE)
                    for k in range(4):
                        g = gg * 4 + k
                        nc.tensor.matmul(ctxg_ps[0:65, 0:128],
                                         lhsT=vaug[:, g * 65:(g + 1) * 65],
                                         rhs=sg_sb[0:64, k * 128:(k + 1) * 128],
                                         start=(g == 0), stop=(g == 63),
                                         skip_group_check=True)
                rec_sb = sb("rec", 128, TT, 2, "grec", dt=F32R)
                with nc.allow_low_precision(reason="f32r rounding of softmax denom"):
                    nc.vector.reciprocal(rec_sb[0:1, 0:128], ctxg_ps[64:65, 0:128])
                bc_ps = acc_ps("gbc")
                nc.tensor.matmul(bc_ps[0:64, 0:128], lhsT=ones_all[0:1, 0:64],
                                 rhs=rec_sb[0:1, 0:128], start=True, stop=True)
                bc_sb = sb("tiny", 64, 128, 3, "gbcs")
                nc.vector.tensor_copy(bc_sb[:], bc_ps[0:64, 0:128])
                gst_sb = sb("tiny", 64, 128, 3, "gst")
                nc.vector.tensor_mul(gst_sb[:], ctxg_ps[0:64, 0:128], bc_sb[:])
                nc.sync.dma_start(
                    bass.AP(gc_loc[l], i * 64 * 128, [[128, 64], [1, 128]]),
                    gst_sb[:])
            nc.gpsimd.collective_compute(
                "AllGather", ALU.bypass, ins=[gc_loc[l][:]],
                outs=[gc_all[l][:]], replica_groups=GROUPS)

            # ---- middle sparse attention ----
            ctxT = pool_b.tile([128, DC * NLOC], F32R, tag="big", name="ctxT")
            for hp in range(6):
                q2_sb = q_all[:, hp * NLOC:(hp + 1) * NLOC]
                ik_sb = sb("ikiv", 128, NSLOT, 4, "ik", dt=I32)
                nc.sync.dma_start(ik_sb[:], idx_k[l, hp])
                iv_sb = sb("ikiv", 128, NSLOT, 4, "iv", dt=I32)
                nc.sync.dma_start(iv_sb[:], idx_v[l, hp])
                ks_sb = sb("gath", 128, NSLOT * 65, 3, "ksel", dt=BF16)
                vs_sb = sb("gath", 128, NSLOT * 65, 3, "vsel", dt=BF16)
                nc.vector.memset(
                    vs_sb[:].rearrange("p (j c) -> p j c", c=65)[:, :, 64:65], 1.0)
                # window slots 1..16 are this core's own blocks 0..15 for
                # every core: fetch from kv_loc at static offsets (HWDGE)
                # instead of 32 per-slot SWDGE gathers
                for hh in range(2):
                    h = 2 * hp + hh
                    nc.sync.dma_start(
                        ks_sb[hh * 64:(hh + 1) * 64, 64:17 * 64].rearrange(
                            "p (b t) -> p b t", t=64),
                        bass.AP(kv_loc[l], h * 16 * 4096,
                                [[64, 64], [4096, 16], [1, 64]]))
                    nc.sync.dma_start(
                        vs_sb[hh * 64:(hh + 1) * 64, 65:17 * 65].rearrange(
                            "p (b c) -> p b c", c=65)[:, :, 0:64],
                        bass.AP(kv_loc[l], (192 + h * 16) * 4096,
                                [[64, 64], [4096, 16], [1, 64]]))
                for j in [0] + list(range(17, NSLOT)):
                    nc.gpsimd.indirect_dma_start(
                        out=ks_sb[:, j * 64:(j + 1) * 64],
                        out_offset=None, in_=kv_rows(l),
                        in_offset=bass.IndirectOffsetOnAxis(
                            ap=ik_sb[:, j:j + 1], axis=0))
                    nc.gpsimd.indirect_dma_start(
                        out=vs_sb[:, j * 65:j * 65 + 64],
                        out_offset=None, in_=kv_rows(l),
                        in_offset=bass.IndirectOffsetOnAxis(
                            ap=iv_sb[:, j:j + 1], axis=0))

                if debug and l == 0 and hp == 0:
                    nc.sync.dma_start(
                        dbg["kva"][:].rearrange("(p c) -> p c", p=128),
                        kv_all[l][:].rearrange("(p c) -> p c", p=128))
                    nc.sync.dma_start(dbg["q2"][:], q2_sb)
                    nc.sync.dma_start(dbg["ksel"][:], ks_sb[:, 0:NSLOT * 64])
                    nc.sync.dma_start(dbg["vsel"][:], vs_sb[:])
                ctx_ps = [[pool_ps.tile([65, 512], F32, tag="ctx", bufs=4,
                                        name=f"ctxps{_hh}{_ha}")
                           for _ha in range(2)] for _hh in range(2)]

                # global slots first: they initialize the ctx accumulators
                for g in (18, 19):
                    for half in range(2):
                        sg_ps = acc_ps("sgG")
                        for hh in range(2):
                            pb = hh * 64
                            nc.tensor.matmul(
                                sg_ps[pb:pb + 64, :],
                                lhsT=ks_sb[pb:pb + 64, g * 64:(g + 1) * 64],
                                rhs=q2_sb[pb:pb + 64, half * 512:(half + 1) * 512],
                                start=True, stop=True)
                        sg_sb = sb("sTsb", 128, TT, 2, "sgGsb", dt=BF16)
                        nc.scalar.activation(sg_sb[:], sg_ps[:], AF.Exp,
                                             scale=SCALE)
                        for hh in range(2):
                            pb = hh * 64
                            nc.tensor.matmul(
                                ctx_ps[hh][half][:],
                                lhsT=vs_sb[pb:pb + 64, g * 65:(g + 1) * 65],
                                rhs=sg_sb[pb:pb + 64, :],
                                start=(g == 18), stop=False,
                                skip_group_check=True)
                # window groups
                for grp in WIN_GROUPS:
                    woff = {}
                    off = 0
                    for s in grp:
                        woff[s] = off
                        off += WIN_W[s]
                    sg_ps = acc_ps("sgW")
                    for hh in range(2):
                        pb = hh * 64
                        for s in grp:
                            nc.tensor.matmul(
                                sg_ps[pb:pb + 64, woff[s]:woff[s] + WIN_W[s]],
                                lhsT=ks_sb[pb:pb + 64, s * 64:(s + 1) * 64],
                                rhs=q2_sb[pb:pb + 64,
                                          WIN_QLO[s] * 64:WIN_QLO[s] * 64 + WIN_W[s]],
                                start=True, stop=True)
                    sg_sb = sb("sTsb", 128, TT, 2, "sgWsb", dt=BF16)
                    nc.scalar.activation(sg_sb[:, 0:off], sg_ps[:, 0:off],
                                         AF.Exp, scale=SCALE)
                    for hh in range(2):
                        pb = hh * 64
                        for s in grp:
                            for half in range(2):
                                qa = max(WIN_QLO[s], half * 8)
                                qb = min(WIN_QHI[s], half * 8 + 7)
                                if qa > qb:
                                    continue
                                nc.tensor.matmul(
                                    ctx_ps[hh][half][:, (qa - half * 8) * 64:(qb + 1 - half * 8) * 64],
                                    lhsT=vs_sb[pb:pb + 64, s * 65:(s + 1) * 65],
                                    rhs=sg_sb[pb:pb + 64,
                                              woff[s] + (qa - WIN_QLO[s]) * 64:
                                              woff[s] + (qb + 1 - WIN_QLO[s]) * 64],
                                    start=False, stop=False,
                                    skip_group_check=True)
                # random groups
                for rg in range(6):
                    sg_ps = acc_ps("sgR")
                    for hh in range(2):
                        pb = hh * 64
                        for kk in range(8):
                            k = rg * 8 + kk
                            j = k // 3
                            nc.tensor.matmul(
                                sg_ps[pb:pb + 64, kk * 64:(kk + 1) * 64],
                                lhsT=ks_sb[pb:pb + 64, (20 + k) * 64:(21 + k) * 64],
                                rhs=q2_sb[pb:pb + 64, j * 64:(j + 1) * 64],
                                start=True, stop=True)
                    sg_sb = sb("sTsb", 128, TT, 2, "sgRsb", dt=BF16)
                    nc.scalar.activation(sg_sb[:], sg_ps[:], AF.Exp, scale=SCALE)
                    for hh in range(2):
                        pb = hh * 64
                        for kk in range(8):
                            k = rg * 8 + kk
                            j = k // 3
                            half, jj = j // 8, j % 8
                            nc.tensor.matmul(
                                ctx_ps[hh][half][:, jj * 64:(jj + 1) * 64],
                                lhsT=vs_sb[pb:pb + 64, (20 + k) * 65:(21 + k) * 65],
                                rhs=sg_sb[pb:pb + 64, kk * 64:(kk + 1) * 64],
                                start=False, stop=(k == 47),
                                skip_group_check=True)
                # normalize + place into ctxT
                for hh in range(2):
                    h = 2 * hp + hh
                    for half in range(2):
                        cps = ctx_ps[hh][half]
                        if debug and l == 0 and hp == 0 and hh == 0 and half == 0:
                            cdump = sb("lnt", 128, TT, 6, "cdump")
                            nc.vector.tensor_copy(cdump[0:65, :], cps[:])
                            nc.sync.dma_start(dbg["cps00"][:], cdump[0:65, :])
                        rec_sb = sb("rec", 128, TT, 2, "rec", dt=F32R)
                        with nc.allow_low_precision(reason="f32r rounding of softmax denom"):
                            nc.vector.reciprocal(rec_sb[0:1, :], cps[64:65, :])
                        bc_ps = acc_ps("bcm")
                        nc.tensor.matmul(bc_ps[0:64, :], lhsT=ones_all[0:1, 0:64],
                                         rhs=rec_sb[0:1, :], start=True, stop=True)
                        bc_sb = sb("bcc", 64, TT, 2, "bcs")
                        nc.vector.tensor_copy(bc_sb[:], bc_ps[0:64, :])
                        nc.vector.tensor_mul(
                            ctxT[(h % 2) * 64:(h % 2) * 64 + 64,
                                 (h // 2) * NLOC + half * 512:
                                 (h // 2) * NLOC + (half + 1) * 512],
                            cps[0:64, :], bc_sb[:])

            # ---- blend global pieces ----
            for jj, jsel in ((0, 0), (15, 1)):
                gc_sb = sb("misc", 128, DC * 64, 3, "gcp")
                for h in range(12):
                    r, i = h // 3, h % 3
                    src = bass.AP(gc_all[l],
                                  r * GCTX_ELEMS + i * 64 * 128 + jsel * 64,
                                  [[128, 64], [1, 64]])
                    nc.sync.dma_start(
                        gc_sb[(h % 2) * 64:(h % 2) * 64 + 64,
                              (h // 2) * 64:(h // 2) * 64 + 64], src)
                ctv = ctxT[:].rearrange("p (c t) -> p c t", t=NLOC)[:, :, jj * 64:(jj + 1) * 64]
                tmp_sb = sb("misc", 128, DC * 64, 3, "gblend")
                nc.vector.tensor_scalar_mul(
                    tmp_sb[:].rearrange("p (c t) -> p c t", t=64), ctv,
                    gmask_sb[:, 2 + jsel:3 + jsel])
                nc.vector.scalar_tensor_tensor(
                    ctv, gc_sb[:].rearrange("p (c t) -> p c t", t=64),
                    gmask_sb[:, jsel:jsel + 1],
                    tmp_sb[:].rearrange("p (c t) -> p c t", t=64),
                    ALU.mult, ALU.add)
            # deferred V bias (valid because softmax weights sum to 1)
            for c in range(DC):
                nc.vector.tensor_scalar_add(
                    ctxT[:, c * NLOC:(c + 1) * NLOC],
                    ctxT[:, c * NLOC:(c + 1) * NLOC],
                    bvv_sb[l][:, c:c + 1])
            if debug and l == 0:
                for c in range(DC):
                    nc.sync.dma_start(dbg["ctx0"][:, c, :],
                                      ctxT[:, c * NLOC:(c + 1) * NLOC])

            # ---- Wo + residual + LN1 ----
            wo_t = load_wtiles("wo", lambda c: w_o[l, c], 768)
            aT = pool_b.tile([128, DC * NLOC], F32R, tag="big", name="aT")
            for tt in range(NTT):
                r_sb = sb("stA", 128, DC * TT, 1, "rwo", dt=F32R)

                def wobias(oc, ps, tt=tt, r_sb=r_sb):
                    nc.vector.scalar_tensor_tensor(
                        r_sb[:, oc * TT:(oc + 1) * TT], ps[:],
                        bo_sb[l][:, oc:oc + 1],
                        xT[:, oc * NLOC + tt * TT:oc * NLOC + (tt + 1) * TT],
                        ALU.add, ALU.add)
                proj_pairs(
                    lambda c: wo_t[c][:],
                    lambda c, tt=tt: ctxT[:, c * NLOC + tt * TT:c * NLOC + (tt + 1) * TT],
                    768, wobias)
                layernorm(lambda c: r_sb[:, c * TT:(c + 1) * TT],
                          ln1g_sb[l], ln1b_sb[l],
                          lambda c: aT[:, c * NLOC + tt * TT:c * NLOC + (tt + 1) * TT])
            if debug and l == 0:
                for c in range(DC):
                    nc.sync.dma_start(dbg["a0"][:, c, :],
                                      aT[:, c * NLOC:(c + 1) * NLOC])

            # ---- FFN + residual + LN2 ----
            xT_next = pool_b.tile([128, DC * NLOC], F32R, tag="big",
                                  name="xT_next")
            for tt in range(NTT):
                r2_sb = sb("stA", 128, DC * TT, 1, "rffn", dt=F32R)
                accs = ([pool_ps.tile([128, TT], F32, tag="acc", bufs=4,
                                      name=f"dpsa{oc}") for oc in range(3)] +
                        [pool_ps.tile([128, TT], F32, tag="ctx", bufs=4,
                                      name=f"dpsc{oc}") for oc in range(3)])
                for fc in range(FFC):
                    hps = pool_ps.tile([128, TT], F32, tag="ctx", bufs=4,
                                       name="hps")
                    wi_t = sb("wistr", 128, 768, 2, "wit", dt=F32R)
                    nc.sync.dma_start(
                        wi_t[:].rearrange("p (c k) -> p c k", k=128),
                        w_i[l, :, :, fc * 128:(fc + 1) * 128].rearrange(
                            "c p k -> p c k"))
                    for c in range(DC):
                        nc.tensor.matmul(
                            hps[:], lhsT=(wi_t[:, c * 128:(c + 1) * 128]),
                            rhs=(aT[:, c * NLOC + tt * TT:c * NLOC + (tt + 1) * TT]),
                            start=(c == 0), stop=(c == DC - 1))
                    hpr_sb = sb("gel", 128, TT, 4, "ghp", dt=F32R)
                    nc.scalar.activation(hpr_sb[:], hps[:], AF.Gelu_apprx_tanh,
                                         bias=bi_sb[l][:, fc:fc + 1])
                    wd_t = sb("wdstr", 128, 768, 2, "wdt", dt=F32R)
                    nc.sync.dma_start(wd_t[:], w_d[l, fc])
                    for oc in range(DC):
                        nc.tensor.matmul(
                            accs[oc][:], lhsT=(wd_t[:, oc * 128:(oc + 1) * 128]),
                            rhs=(hpr_sb[:]),
                            start=(fc == 0), stop=(fc == FFC - 1),
                            skip_group_check=True)
                for oc in range(DC):
                    nc.vector.scalar_tensor_tensor(
                        r2_sb[:, oc * TT:(oc + 1) * TT], accs[oc][:],
                        bd_sb[l][:, oc:oc + 1],
                        aT[:, oc * NLOC + tt * TT:oc * NLOC + (tt + 1) * TT],
                        ALU.add, ALU.add)
                layernorm(lambda c: r2_sb[:, c * TT:(c + 1) * TT],
                          ln2g_sb[l], ln2b_sb[l],
                          lambda c: xT_next[:, c * NLOC + tt * TT:c * NLOC + (tt + 1) * TT])
            xT = xT_next
            if debug and l == 0:
                for c in range(DC):
                    nc.sync.dma_start(dbg["x1"][:, c, :],
                                      xT[:, c * NLOC:(c + 1) * NLOC])

        # ---------- pooled mean + fc ----------
        xsum_sb = sb("tiny", 128, DC, 3, "xsum")
        for c in range(DC):
            nc.vector.reduce_sum(xsum_sb[:, c:c + 1],
                                 xT[:, c * NLOC:(c + 1) * NLOC],
                                 axis=mybir.AxisListType.X)
        fc_ps = ctx_ps_t("fcps")
        for c in range(DC):
            nc.tensor.matmul(fc_ps[0:1, 0:1], lhsT=xsum_sb[:, c:c + 1],
                             rhs=fcw_sb[:, c:c + 1],
                             start=(c == 0), stop=(c == DC - 1))
        ofc_sb = sb("tiny2", 1, 1, 2, "ofc")
        nc.vector.tensor_copy(ofc_sb[:], fc_ps[0:1, 0:1])
        nc.sync.dma_start(out_fc[:], ofc_sb[:])

    nc.compile()
    return nc


# ======================= host side =======================

def _vec128(v):
    return np.ascontiguousarray(np.asarray(v, np.float32).reshape(DC, 128).T)


def _slot_k(g, h):
    return (g // 16) * SLOTS_RANK + h * 16 + (g % 16)


def _slot_v(g, h):
    return (g // 16) * SLOTS_RANK + 192 + h * 16 + (g % 16)


def _make_gindices(c):
    igk = np.zeros((L, 3, 64, 64), np.int32)
    igv = np.zeros((L, 3, 64, 64), np.int32)
    p = np.arange(64)
    for l in range(L):
        for i in range(3):
            h = 3 * c + i
            sk = np.array([_slot_k(g, h) for g in range(64)])
            sv = np.array([_slot_v(g, h) for g in range(64)])
            igk[l, i] = sk[None, :] * 64 + p[:, None]
            igv[l, i] = sv[None, :] * 64 + p[:, None]
    return igk, igv


def _make_indices(rand_blocks, c):
    ik = np.zeros((L, 6, 128, NSLOT), np.int32)
    iv = np.zeros((L, 6, 128, NSLOT), np.int32)
    p = np.arange(64)
    for l in range(L):
        for hp in range(6):
            for hh in range(2):
                h = 2 * hp + hh
                gs = np.zeros(NSLOT, np.int64)
                for s in range(18):
                    gs[s] = min(max(c * 16 - 1 + s, 0), 63)
                gs[18], gs[19] = 0, 63
                for j in range(16):
                    m = c * 16 + j
                    for r_ in range(R):
                        gs[20 + j * 3 + r_] = (rand_blocks[l, h, m, r_]
                                               if 1 <= m <= 62 else 0)
                sk = np.array([_slot_k(g, h) for g in gs])
                sv = np.array([_slot_v(g, h) for g in gs])
                ik[l, hp, hh * 64:(hh + 1) * 64, :] = sk[None, :] * 64 + p[:, None]
                iv[l, hp, hh * 64:(hh + 1) * 64, :] = sv[None, :] * 64 + p[:, None]
    return ik, iv


_CACHE = {}


def _get_nc(debug=False):
    key = "dbg" if debug else "plain"
    if key not in _CACHE:
        _CACHE[key] = build(debug)
    return _CACHE[key]


def make_in_maps(inputs):
    f32 = lambda a: np.ascontiguousarray(np.asarray(a, np.float32))
    inp = {k: np.asarray(v) for k, v in inputs.items()}
    w_qkv = f32(np.concatenate([inp["Wq"], inp["Wk"], inp["Wv"]], axis=2)
                .reshape(L, DC, 128, 2304))
    w_o = f32(inp["Wo"].reshape(L, DC, 128, D))
    w_i = f32(inp["Wi"].reshape(L, DC, 128, FF))
    w_d = f32(np.asarray(inp["Wd"], np.float32).reshape(L, FFC, 128, D))
    b_qk = f32(np.stack([np.concatenate(
        [_vec128(inp["bq"][l]), _vec128(inp["bk"][l])], axis=1)
        for l in range(L)]))
    b_v = f32(np.stack([_vec128(inp["bv"][l]) for l in range(L)]))
    b_o = f32(np.stack([_vec128(inp["bo"][l]) for l in range(L)]))
    b_i = f32(np.stack([np.ascontiguousarray(
        np.asarray(inp["bi"][l], np.float32).reshape(FFC, 128).T)
        for l in range(L)]))
    b_d = f32(np.stack([_vec128(inp["bd"][l]) for l in range(L)]))
    emb_g, emb_b = _vec128(inp["emb_ln_g"]), _vec128(inp["emb_ln_b"])
    ln1_g = f32(np.stack([_vec128(inp["ln1_g"][l]) for l in range(L)]))
    ln1_b = f32(np.stack([_vec128(inp["ln1_b"][l]) for l in range(L)]))
    ln2_g = f32(np.stack([_vec128(inp["ln2_g"][l]) for l in range(L)]))
    ln2_b = f32(np.stack([_vec128(inp["ln2_b"][l]) for l in range(L)]))
    fc_w = _vec128(inp["fc_w"][:, 0])
    pos_tt = f32(inp["pos_emb"] + np.asarray(inp["tt_emb"])[None, :])
    emb = f32(inp["inputs_embeds"])
    rand_blocks = np.asarray(inp["rand_blocks"])

    idx_cache = {}
    in_maps = []
    for core in range(8):
        b, c = core // 4, core % 4
        rows = slice(c * NLOC, (c + 1) * NLOC)
        xT0 = np.ascontiguousarray(
            emb[b, rows].T.reshape(DC, 128, NLOC).transpose(1, 0, 2))
        posTa = np.ascontiguousarray(
            pos_tt[rows].T.reshape(DC, 128, NLOC).transpose(1, 0, 2))
        if c not in idx_cache:
            idx_cache[c] = _make_indices(rand_blocks, c) + _make_gindices(c)
        ik, iv, igk, igv = idx_cache[c]
        m0, m15 = float(c == 0), float(c == 3)
        gm = np.zeros((128, 4), np.float32)
        gm[:, 0], gm[:, 1], gm[:, 2], gm[:, 3] = m0, m15, 1 - m0, 1 - m15
        in_maps.append({
            "xT0": xT0, "posT": posTa, "w_qkv": w_qkv, "w_o": w_o,
            "w_i": w_i, "w_d": w_d, "b_qk": b_qk, "b_v": b_v,
            "b_o": b_o, "b_i": b_i, "b_d": b_d, "emb_g": emb_g,
            "emb_b": emb_b, "ln1_g": ln1_g, "ln1_b": ln1_b, "ln2_g": ln2_g,
            "ln2_b": ln2_b, "fc_w": fc_w, "idx_k": ik, "idx_v": iv,
            "gmask": gm, "hbase": np.array([[3 * c]], np.uint32),
            "ones_in": np.ones((128, 128), np.float32),
            "idx_gk": igk, "idx_gv": igv,
        })
    return in_maps


def finish(inputs, results):
    fc_b = float(np.asarray(inputs["fc_b"])[0])
    out = np.zeros(B, np.float32)
    for b in range(B):
        tot = sum(float(results[4 * b + c]["out_fc"][0, 0]) for c in range(4))
        out[b] = tot / N + fc_b
    return out.astype(np.float32)


class _Runner:
    """Cached shard_map-jitted executable over the 8 NeuronCores.

    Mirrors bass2jax.run_bass_via_pjrt's multi-core path, but caches the
    jitted callable so repeated kernel() calls don't retrace/recompile, and
    exposes device-resident-input execution for timing.
    """

    def __init__(self, nc):
        import jax
        import concourse.mybir as mybir_
        from concourse import bass2jax
        from jax.sharding import Mesh, PartitionSpec, NamedSharding
        bass2jax.install_neuronx_cc_hook()
        self.jax = jax
        in_names, out_names, out_avals = [], [], []
        pname = nc.partition_id_tensor.name if nc.partition_id_tensor else None
        for alloc in nc.m.functions[0].allocations:
            if not isinstance(alloc, mybir_.MemoryLocationSet):
                continue
            name = alloc.memorylocations[0].name
            if alloc.kind == "ExternalInput":
                if name != pname:
                    in_names.append(name)
            elif alloc.kind == "ExternalOutput":
                out_names.append(name)
                out_avals.append(jax.core.ShapedArray(
                    tuple(alloc.tensor_shape), mybir_.dt.np(alloc.dtype)))
        self.in_names, self.out_names, self.out_avals = in_names, out_names, out_avals
        n_params, n_outs = len(in_names), len(out_avals)
        all_names = in_names + out_names
        if pname is not None:
            all_names.append(pname)

        def _body(*args):
            operands = list(args)
            if pname is not None:
                operands.append(bass2jax.partition_id_tensor())
            outs = bass2jax._bass_exec_p.bind(
                *operands, out_avals=tuple(out_avals),
                in_names=tuple(all_names), out_names=tuple(out_names),
                lowering_input_output_aliases=(),
                sim_require_finite=True, sim_require_nnan=True, nc=nc)
            return tuple(outs)

        from jax.experimental.shard_map import shard_map
        devices = jax.devices()[:8]
        self.mesh = Mesh(np.asarray(devices), ("core",))
        in_specs = (PartitionSpec("core"),) * (n_params + n_outs)
        out_specs = (PartitionSpec("core"),) * n_outs
        self.sharding = NamedSharding(self.mesh, PartitionSpec("core"))
        self.fn = jax.jit(shard_map(_body, mesh=self.mesh, in_specs=in_specs,
                                    out_specs=out_specs, check_rep=False),
                          keep_unused=True)
        self.n_params, self.n_outs = n_params, n_outs

    def device_args(self, in_maps):
        jax = self.jax
        concat_in = [np.concatenate([np.asarray(in_maps[c][n])
                                     for c in range(8)], axis=0)
                     for n in self.in_names]
        concat_zero = [np.zeros((8 * a.shape[0], *a.shape[1:]), a.dtype)
                       for a in self.out_avals]
        return [jax.device_put(a, self.sharding)
                for a in concat_in + concat_zero]

    def run_device(self, dargs):
        outs = self.fn(*dargs)
        self.jax.block_until_ready(outs)
        return outs

    def run(self, in_maps):
        outs = self.run_device(self.device_args(in_maps))
        res = []
        for c in range(8):
            res.append({n: np.asarray(outs[i]).reshape(
                8, *self.out_avals[i].shape)[c]
                for i, n in enumerate(self.out_names)})
        return res


def _get_runner():
    if "runner" not in _CACHE:
        _CACHE["runner"] = _Runner(_get_nc())
    return _CACHE["runner"]


def kernel(**inputs):
    runner = _get_runner()
    in_maps = make_in_maps(inputs)
    return finish(inputs, runner.run(in_maps))

